# revision 22
# baseline (speedup 1.0000x reference)
"""Trainium2 Bass kernel for CombinedVectorField (CFG vector field + exact
Jacobian-trace divergence).

Math: with u = tanh(x@W1x + h@W1h + b1'), b1' = b1 + t*W1[256],
  v(x,h)  = u @ W2 + b2
  div(x,h)= sum_k (1-u_k^2) c_k = d0 - (u*u) @ c,   c_k = sum_i W1x[i,k] W2[k,i]
Output = concat[(1-gs)*v_null + gs*v_h, (1-gs)*div_null + gs*div_h].

Sharding: pure data parallel - each of the 8 cores takes 512 batch rows
(both guidance branches), weights replicated, feature-major layouts.

Schedule notes (v4):
- Inputs split into 6 need-ordered pieces over the two HWDGE rings so the
  chain rarely stalls even at ~110 GB/s/ring (per-ring rate varies run to
  run with HBM neighbor traffic):
  ring A (scalar): [xa|w1x01|w1h01|aux], [w1x23|w1h23], [w2*gs|cm*gs]
  ring B (sync):   [ha|hna], [xb|hb|hnb], [w2*(1-gs)|cm*(1-gs)]
  Ring FIFO = need order; issues are the first body instructions.
- PE-HAM: the un-throttle to 2.4 GHz needs one fully-busy 3.4us window,
  and any DMA-wait gap restarts it.  12 back-to-back prewarm matmuls
  bridge body entry to the first L1 inputs, and single filler matmuls
  after the first L1 groups absorb per-piece DMA jitter so the busy
  window stays unbroken until the flip.
- Each (chunk, half) L1 output is one [128,512] PSUM bank holding both
  guidance branches side by side -> one tanh ACTIVATE per group with a
  per-partition f32 chunk bias (bf16-shipped, converted once on DVE).
- The tanh chain runs [c0a c1a c2a c0b c1b c2b c3b c3a]; all L2 matmuls
  are back-loaded after the L1 stream (they drain at full PE rate behind
  mostly-finished tanhs).  Tail: u2(3,0) is emitted as two [128,256]
  halves ahead of voutb on DVE so pdiv-a can start right after the last
  tanh; pdc is split per half so the DO DMA issues ~1.5us earlier than
  a fused tail would.
- Divergence matmuls are column-tiled 4-way (tile_position=(0,32c)):
  four chunk partials run concurrently into distinct partitions of one
  bank and ship as 4 bf16 rows summed on the host with d0.
- Teardown: the body-end drain still waits for output-DMA completion and
  the sem-only barrier is kept, but the gpsimd dma_reset/sem_clear pair
  is dropped - the runtime postamble clears the whole semaphore file
  anyway.
"""
import sys

sys.path.insert(0, "/opt/trn_rl_repo")

import ml_dtypes
import numpy as np

import concourse.bass as bass
import concourse.tile as tile
from concourse import bacc, mybir
from concourse.bass_utils import run_bass_kernel_spmd
from concourse.vector_clock import ScopedClock


class _TrimTileContext(tile.TileContext):
    """TileContext with the final all-engine barrier dropped from the
    teardown and the mid barrier reduced to sem-only (no per-engine
    drains). The head drain still waits for every semaphore (incl.
    output-DMA completion) and semaphores are still cleared for the next
    execution; only the trailing barrier (nothing executes after it) is
    elided."""

    def _drain_and_barrier(self, tick_clock, wait_clock):
        drain_inst = self.nc.sync.drain()
        wait_clock.add_sem_waits(
            drain_inst.ins, ScopedClock({None: tick_clock.global_clock})
        )
        self.nc.all_engine_barrier(sem_only=True)
        popped = self.nc._tile_sem_poison_stack.pop()
        assert popped is self._sem_poison
        self.nc.clear_and_free_semaphores(list(self.sems.allocated().values()))


class _FastBacc(bacc.Bacc):
    """Bacc whose constructor-time all-engine barrier (after the const-tile
    memsets) is sem-only - the per-engine drains there cost ~1us of kernel
    head time and order nothing we rely on beyond the memsets, which the
    event-semaphore barrier already orders."""

    def all_engine_barrier(self, *, sem_only: bool = False):
        super().all_engine_barrier(sem_only=True)

F32 = mybir.dt.float32
BF16 = mybir.dt.bfloat16
AF = mybir.ActivationFunctionType
ALU = mybir.AluOpType

N_CORES = 8
B = 4096
DIM_X = 128
DIM_H = 128
HIDDEN = 512
R = B // N_CORES          # rows per core
HR = R // 2               # rows per half
NCH = HIDDEN // 128       # hidden chunks

NA1 = HR + 512 + 5        # xa | w1x0 w1h0 w1x1 w1h1 | aux
NA2 = 512                 # w1x2 w1h2 w1x3 w1h3
NA3 = NCH * DIM_X + NCH   # w2*gs | cm*gs
NB1 = 2 * HR              # ha | hna
NB2 = 3 * HR              # xb | hb | hnb
NB3 = NCH * DIM_X + NCH   # w2*(1-gs) | cm*(1-gs)

_NC_CACHE = None


def _build():
    nc = _FastBacc("TRN2", target_bir_lowering=False, debug=False,
                   enable_asserts=False, monotonic_sem_count=0)

    inA1 = nc.dram_tensor("inA1", [128, NA1], BF16, kind="ExternalInput")
    inA2 = nc.dram_tensor("inA2", [128, NA2], BF16, kind="ExternalInput")
    inA3 = nc.dram_tensor("inA3", [128, NA3], BF16, kind="ExternalInput")
    inB1 = nc.dram_tensor("inB1", [128, NB1], BF16, kind="ExternalInput")
    inB2 = nc.dram_tensor("inB2", [128, NB2], BF16, kind="ExternalInput")
    inB3 = nc.dram_tensor("inB3", [128, NB3], BF16, kind="ExternalInput")

    VO = nc.dram_tensor("VO", [DIM_X, R], BF16, kind="ExternalOutput")
    DO = nc.dram_tensor("DO", [NCH, R], BF16, kind="ExternalOutput")

    with _TrimTileContext(nc) as tc:
        with tc.tile_pool(name="cst", bufs=1) as cst, \
             tc.tile_pool(name="act", bufs=8) as actp, \
             tc.tile_pool(name="out", bufs=1) as outp, \
             tc.tile_pool(name="psg", bufs=4, space="PSUM") as psg, \
             tc.tile_pool(name="psv", bufs=1, space="PSUM") as psv:
            # input DMA issues first - ring FIFO enforces need order.
            a1 = cst.tile([128, NA1], BF16)
            nc.scalar.dma_start(out=a1[:], in_=inA1[:])
            b1t = cst.tile([128, NB1], BF16)
            nc.sync.dma_start(out=b1t[:], in_=inB1[:])
            # dummy activation: pulls the ~1.3us ACT table load ahead,
            # overlapping A1's transfer instead of delaying the first tanh.
            warmact = outp.tile([128, 1], F32)
            nc.scalar.activation(warmact[:], nc.const_aps.aps[(F32, 0.0)],
                                 AF.Tanh, bias=0.0, scale=1.0)
            a2 = cst.tile([128, NA2], BF16)
            nc.scalar.dma_start(out=a2[:], in_=inA2[:])
            b2t = cst.tile([128, NB2], BF16)
            nc.sync.dma_start(out=b2t[:], in_=inB2[:])
            a3 = cst.tile([128, NA3], BF16)
            nc.scalar.dma_start(out=a3[:], in_=inA3[:])
            b3t = cst.tile([128, NB3], BF16)
            nc.sync.dma_start(out=b3t[:], in_=inB3[:])

            # PE prewarm: back-to-back cold matmuls keep the PE-HAM busy
            # window unbroken from kernel entry until the first L1 inputs
            # land.
            # prewarm output is garbage in the pda bank, fully overwritten
            # by the start=True divergence matmuls
            wrm = cst.tile([128, 256], BF16)
            nc.gpsimd.memset(wrm[:], 0.0)
            pwarm = psv.tile([128, 256], F32, tag="pda")

            def filler(n):
                for _ in range(n):
                    nc.tensor.matmul(pwarm[:], wrm[:, 0:128], wrm[:],
                                     start=True, stop=True,
                                     skip_group_check=True)

            filler(13)

            # bf16 -> f32 bias conversion (one cheap DVE op once A1 lands)
            f32aux = cst.tile([128, 5], F32)
            nc.vector.tensor_copy(f32aux[:], a1[:, HR + 512:NA1])

            xa = a1[:, 0:HR]
            ha = b1t[:, 0:HR]
            hna = b1t[:, HR:2 * HR]
            xb = b2t[:, 0 * HR:1 * HR]
            hb = b2t[:, 1 * HR:2 * HR]
            hnb = b2t[:, 2 * HR:3 * HR]

            def w1x(c):
                return [a1[:, HR:HR + 128], a1[:, HR + 256:HR + 384],
                        a2[:, 0:128], a2[:, 256:384]][c]

            def w1h(c):
                return [a1[:, HR + 128:HR + 256], a1[:, HR + 384:HR + 512],
                        a2[:, 128:256], a2[:, 384:512]][c]

            def w2c(c, br):
                t = a3 if br == 0 else b3t
                return t[:, 128 * c:128 * (c + 1)]

            def cmc(c, br):
                t = a3 if br == 0 else b3t
                return t[:, NCH * DIM_X + c:NCH * DIM_X + c + 1]

            pva = psv.tile([128, HR], F32, tag="pva")
            pvb = psv.tile([128, HR], F32, tag="pvb")
            # per-half divergence banks: separate tiles so the a-half
            # matmuls never serialize behind the b-half evacuation copy
            pda = psv.tile([128, HR], F32, tag="pda")
            pdb = psv.tile([128, HR], F32, tag="pdb")
            pdh = {0: pda, 1: pdb}

            ut = {}
            u2t = {}
            gt = {}

            def l1(c, half, x_, h_, hn_):
                g = psg.tile([128, 2 * HR], F32, tag="g")
                gt[(c, half)] = g
                nc.tensor.matmul(g[:, 0:HR], w1x(c), x_, start=True, stop=False)
                nc.tensor.matmul(g[:, HR:2 * HR], w1x(c), x_, start=False, stop=False)
                nc.tensor.matmul(g[:, 0:HR], w1h(c), h_, start=False, stop=False)
                nc.tensor.matmul(g[:, HR:2 * HR], w1h(c), hn_, start=False, stop=True)

            def act(c, half, defer_u2=False):
                u = actp.tile([128, 2 * HR], BF16, tag="u")
                ut[(c, half)] = u
                nc.scalar.activation(u[:], gt[(c, half)][:], AF.Tanh,
                                     bias=f32aux[:, c:c + 1], scale=1.0)
                if not defer_u2:
                    emit_u2(c, half)

            def emit_u2(c, half):
                u2 = actp.tile([128, 2 * HR], BF16, tag="u2")
                u2t[(c, half)] = u2
                nc.vector.tensor_tensor(u2[:], ut[(c, half)][:], ut[(c, half)][:],
                                        op=ALU.mult)

            def emit_u2_halves(c, half):
                # two [128,256] squares: the h-branch half lands first so
                # pdiv(br=0) can start before the null half finishes.
                u2 = actp.tile([128, 2 * HR], BF16, tag="u2")
                u2t[(c, half)] = u2
                nc.vector.tensor_tensor(u2[:, 0:HR], ut[(c, half)][:, 0:HR],
                                        ut[(c, half)][:, 0:HR], op=ALU.mult)
                nc.vector.tensor_tensor(u2[:, HR:2 * HR], ut[(c, half)][:, HR:2 * HR],
                                        ut[(c, half)][:, HR:2 * HR], op=ALU.mult)

            def l2(c, half, pv, first, last):
                u = ut[(c, half)]
                nc.tensor.matmul(pv[:], w2c(c, 0), u[:, 0:HR],
                                 start=first, stop=False)
                nc.tensor.matmul(pv[:], w2c(c, 1), u[:, HR:2 * HR],
                                 start=False, stop=last)

            def pdiv(br, half, first, last):
                # 4 chunk partials run concurrently on distinct col groups,
                # landing at partitions {0,32,64,96} of the half's pd bank.
                pdt = pdh[half]
                us = slice(0, HR) if br == 0 else slice(HR, 2 * HR)
                for c in range(NCH):
                    nc.tensor.matmul(pdt[32 * c:32 * c + 1, :], cmc(c, br),
                                     u2t[(c, half)][:, us],
                                     start=first,
                                     stop=(last and c == NCH - 1),
                                     tile_position=(0, 32 * c))

            # chain order: b-half accumulator closes at tanh #7, a-half at
            # tanh #8 - only the small divergence DMA tails the last tanh.
            CHAIN = [(0, 0), (1, 0), (2, 0), (0, 1), (1, 1), (2, 1), (3, 1), (3, 0)]
            for c, half in CHAIN:
                if half == 0:
                    l1(c, 0, xa, ha, hna)
                else:
                    l1(c, 1, xb, hb, hnb)
                act(c, half, defer_u2=((c, half) == (3, 0)))
                # single filler matmuls absorb per-piece DMA jitter so the
                # HAM busy window stays unbroken before the 2.4 GHz flip
                if (c, half) == (0, 0):
                    filler(1)
                if (c, half) == (1, 0):
                    filler(2)
                if (c, half) == (2, 0):
                    filler(1)
                # pva's first chunks ride the ACT-paced slack of the b-half
                # chain (their tanhs and the W2 pieces are long landed), so
                # the PE tail after the last tanh stays short.
                if (c, half) == (1, 1):
                    l2(0, 0, pva, True, False)
                if (c, half) == (2, 1):
                    l2(1, 0, pva, False, False)
                if (c, half) == (3, 1):
                    l2(2, 0, pva, False, False)

            l2(0, 1, pvb, True, False)
            l2(1, 1, pvb, False, False)
            l2(2, 1, pvb, False, False)
            l2(3, 1, pvb, False, True)
            # the last chunk's squares get the DVE right after the final
            # tanh; voutb rides the ACT engine's idle window between the
            # last tanh and vouta, so neither evacuation blocks pdiv-a.
            emit_u2_halves(3, 0)
            voutb = outp.tile([128, HR], BF16)
            nc.scalar.activation(voutb[:], pvb[:], AF.Identity,
                                 bias=f32aux[:, 4:5], scale=1.0)
            nc.sync.dma_start(out=VO[:, HR:R], in_=voutb[:])
            pdiv(0, 1, True, False)
            pdiv(1, 1, False, True)
            pdc = outp.tile([128, R], BF16)
            nc.vector.tensor_copy(pdc[:, HR:R], pdh[1][:])
            l2(3, 0, pva, False, True)
            pdiv(0, 0, True, False)
            pdiv(1, 0, False, True)

            # v half-a on ACT (free right after the last tanh), out on the
            # scalar ring; divergence partials: per-half bank copies -> one
            # 4-row DMA on sync, summed on the host.
            vouta = outp.tile([128, HR], BF16)
            nc.scalar.activation(vouta[:], pva[:], AF.Identity,
                                 bias=f32aux[:, 4:5], scale=1.0)
            nc.scalar.dma_start(out=VO[:, 0:HR], in_=vouta[:])
            nc.vector.tensor_copy(pdc[:, 0:HR], pdh[0][:])
            nc.sync.dma_start(out=DO[:], in_=pdc[0:97:32, :])
    nc.compile()
    return nc


def _get_nc():
    global _NC_CACHE
    if _NC_CACHE is None:
        _NC_CACHE = _build()
    return _NC_CACHE


def _prep_in_maps(state, h, h_null, t, guidance_scale, W1, b1, W2, b2):
    f32 = np.float32
    bf = ml_dtypes.bfloat16
    xTf = state[:, :DIM_X].T.astype(bf)                            # (128, B)
    hTf = h.T.astype(bf)
    hnTf = h_null.T.astype(bf)
    w1xf = W1[:DIM_X].astype(bf)                                   # (128, 512)
    w1hf = W1[DIM_X:DIM_X + DIM_H].astype(bf)
    b1p = (b1.astype(f32) + t.astype(f32)[0] * W1[DIM_X + DIM_H].astype(f32))
    w2r = W2.astype(f32).reshape(NCH, 128, DIM_X).transpose(1, 0, 2).reshape(128, NCH * DIM_X)
    cvec = (W1[:DIM_X].astype(np.float64) * W2.astype(np.float64).T).sum(0)  # (512,)
    d0 = float(cvec.sum())
    cmatf = cvec.reshape(NCH, 128).T.astype(f32)                   # (128, NCH)
    gs = float(guidance_scale.astype(f32)[0])

    auxf = np.zeros((128, 5), f32)
    auxf[:, 0:4] = b1p.reshape(NCH, 128).T
    auxf[:, 4] = b2.astype(f32)
    auxbf = auxf.astype(bf)

    inA2 = np.ascontiguousarray(
        np.concatenate([w1xf[:, 256:384], w1hf[:, 256:384],
                        w1xf[:, 384:512], w1hf[:, 384:512]], axis=1))
    inA3 = np.ascontiguousarray(
        np.concatenate([gs * w2r, -gs * cmatf], axis=1).astype(bf))
    inB3 = np.ascontiguousarray(
        np.concatenate([(1.0 - gs) * w2r, -(1.0 - gs) * cmatf],
                       axis=1).astype(bf))

    in_maps = []
    for i in range(N_CORES):
        sl_a = slice(i * R, i * R + HR)
        sl_b = slice(i * R + HR, (i + 1) * R)
        in_maps.append({
            "inA1": np.ascontiguousarray(
                np.concatenate([xTf[:, sl_a], w1xf[:, 0:128], w1hf[:, 0:128],
                                w1xf[:, 128:256], w1hf[:, 128:256],
                                auxbf], axis=1)),
            "inA2": inA2,
            "inA3": inA3,
            "inB1": np.ascontiguousarray(
                np.concatenate([hTf[:, sl_a], hnTf[:, sl_a]], axis=1)),
            "inB2": np.ascontiguousarray(
                np.concatenate([xTf[:, sl_b], hTf[:, sl_b], hnTf[:, sl_b]],
                               axis=1)),
            "inB3": inB3,
        })
    return in_maps, d0


def kernel(state, h, h_null, t, guidance_scale, W1, b1, W2, b2, _trace=False):
    nc = _get_nc()
    in_maps, d0 = _prep_in_maps(state, h, h_null, t, guidance_scale,
                                W1, b1, W2, b2)
    res = run_bass_kernel_spmd(nc, in_maps, list(range(N_CORES)), trace=_trace)
    out = np.empty((B, DIM_X + 1), np.float32)
    for i in range(N_CORES):
        sl = slice(i * R, (i + 1) * R)
        out[sl, :DIM_X] = res.results[i]["VO"].astype(np.float32).T
        out[sl, DIM_X] = res.results[i]["DO"].astype(np.float32).sum(0) + d0
    if _trace:
        return out, res
    return out
